# revision 23
# baseline (speedup 1.0000x reference)
"""GatedGCN Trainium2 kernel — 8-core SPMD, self-contained.

Strategy
--------
dst-shard the graph across 8 NeuronCores. Node features live in DRAM as an
fp16 table with 4 nodes packed per 256B row, so the bulk `dma_gather`
(int16 indices, 256B elements) can fetch `h[src]` for every edge in one
index window. Nodes are relabeled per shard by descending in-degree and
grouped into 128-node panels; each edge occupies a (node, slot) cell, so
the weighted segment-sum becomes a DVE multiply over the panel followed by
a fused scalar_tensor_tensor add-tree (4x DVE mode) over the slots (no
scatter anywhere). Layer hand-off between cores is a single fp16
AllGather. The GRU runs feature-major in fp16 with biases folded into
per-partition scalar APs of fused stt ops; W_nn is folded into W_ih
(gi = (Wih@Wnn)agg + Wih@b_nn + b_ih). The 2-class log_softmax head is
computed as -softplus(+/-d) from a single logit-difference matmul.
"""
import sys

sys.path.insert(0, "/opt/trn_rl_repo")

import numpy as np

import concourse.bacc as bacc
import concourse.bass as bass
import concourse.mybir as mybir
import concourse.tile as tile
from concourse.bass_utils import run_bass_kernel_spmd
from concourse.masks import make_identity

N = 100000
E = 1600000
H = 32
NCLS = 2
LAYERS = 2
NCORES = 8
KCH = 8  # slot-groups (128 idxs each) per dma_gather call; scratch = 1024*KCH

F32 = mybir.dt.float32
F16 = mybir.dt.float16
I16 = mybir.dt.int16
AF = mybir.ActivationFunctionType
ALU = mybir.AluOpType


def _split_multiwaits(nc, max_waits=1):
    """This walrus build rejects >1 sync-wait per instruction; split extras
    onto same-engine InstNoOp predecessors (semantically identical)."""
    ctr = 0
    for fn in nc.m.functions:
        for bb in fn.blocks:
            new_insts = []
            for inst in bb.instructions:
                si = inst.sync_info
                waits = list(si.on_wait) if si is not None and si.on_wait else []
                if len(waits) > max_waits:
                    head, tail = waits[:-max_waits], waits[-max_waits:]
                    for i in range(0, len(head), max_waits):
                        ctr += 1
                        nop = mybir.InstNoOp(name=f"WSPLIT-{ctr}", engine=inst.engine)
                        nop.sync_info = mybir.SyncInfo(
                            on_wait=head[i : i + max_waits], on_update=[]
                        )
                        nc.register_instruction(nop, overwrite=True)
                        new_insts.append(nop)
                    inst.sync_info = mybir.SyncInfo(
                        on_wait=tail,
                        on_update=list(si.on_update) if si.on_update else [],
                    )
                new_insts.append(inst)
            bb.instructions[:] = new_insts


def _sizes(n):
    shard = n // NCORES
    shard_pad = -(-shard // 512) * 512
    panels = shard_pad // 128
    tabrows = NCORES * shard_pad // 4
    return shard, shard_pad, panels, tabrows


def _preprocess(edge_index, edge_weight):
    shard, shard_pad, panels, tabrows = _sizes(N)
    src_ = np.asarray(edge_index[0], dtype=np.int64)
    dst = np.asarray(edge_index[1], dtype=np.int64)
    src = src_
    w = np.asarray(edge_weight, dtype=np.float32)

    deg = np.bincount(dst, minlength=N)
    shards = np.arange(N) // shard
    order = np.lexsort((np.arange(N), -deg, shards))  # old ids by (shard, -deg)
    new_of_old = np.empty(N, dtype=np.int64)
    pos = np.arange(N)
    c_of_pos = pos // shard
    r_of_pos = pos - c_of_pos * shard
    new_of_old[order] = c_of_pos * shard_pad + r_of_pos

    s_new = new_of_old[src]
    d_new = new_of_old[dst]
    core = d_new // shard_pad
    r = d_new % shard_pad
    # slot index k per edge: occurrence number among edges sharing the dst
    eorder = np.argsort(d_new, kind="stable")
    ds = d_new[eorder]
    starts = np.r_[0, np.nonzero(np.diff(ds))[0] + 1]
    counts = np.diff(np.r_[starts, len(ds)])
    k_sorted = np.arange(len(ds)) - np.repeat(starts, counts)
    k = np.empty(src_.size, dtype=np.int64)
    k[eorder] = k_sorted

    # per-core per-panel K, unified across cores (SPMD: one program)
    deg_new = np.zeros(NCORES * shard_pad, dtype=np.int64)
    deg_new[new_of_old] = deg
    K_uni = np.zeros(panels, dtype=np.int64)
    for c in range(NCORES):
        base = c * shard_pad
        firsts = deg_new[base : base + shard_pad : 128]  # max of each panel
        K_uni = np.maximum(K_uni, firsts)
    K_uni = K_uni.astype(np.int64)
    sumK = int(K_uni.sum())
    col0 = np.zeros(panels + 1, dtype=np.int64)
    col0[1:] = np.cumsum(128 * K_uni)
    slots_total = int(col0[-1])

    p_of_edge = r // 128
    q_of_edge = r % 128
    slotpos = col0[p_of_edge] + k * 128 + q_of_edge

    idx_imgs = np.zeros((NCORES, 128, 8 * sumK), dtype=np.int16)
    w4_imgs = np.zeros((NCORES, 128, 4 * sumK), dtype=np.float16)
    for c in range(NCORES):
        m = core == c
        ia = np.zeros(slots_total, dtype=np.int16)
        wa = np.zeros(slots_total * 4, dtype=np.float16)
        ia[slotpos[m]] = (s_new[m] >> 2).astype(np.int16)
        wa[slotpos[m] * 4 + (s_new[m] & 3)] = w[m].astype(np.float16)
        icols = 0
        wcols = 0
        for p in range(panels):
            K = int(K_uni[p])
            if K == 0:
                continue
            a, b = int(col0[p]), int(col0[p + 1])
            blk = ia[a:b].reshape(K * 8, 16).T  # [16, 8K]
            idx_imgs[c, :, icols : icols + 8 * K] = np.tile(blk, (8, 1))
            wb = wa[4 * a : 4 * b].reshape(K, 128, 4)
            w4_imgs[c, :, wcols : wcols + 4 * K] = wb.transpose(1, 0, 2).reshape(
                128, 4 * K
            )
            icols += 8 * K
            wcols += 4 * K
    return {
        "order": order,
        "K_uni": K_uni,
        "idx_imgs": idx_imgs,
        "w4_imgs": w4_imgs,
        "sumK": sumK,
    }


_BUILD_CACHE = {}


def _build(K_uni, fuse):
    key = (tuple(int(x) for x in K_uni), tuple(float(x) for x in fuse))
    if key in _BUILD_CACHE:
        return _BUILD_CACHE[key]

    shard, shard_pad, panels, tabrows = _sizes(N)
    sumK = int(np.sum(K_uni))
    batches = panels // 4

    nc = bacc.Bacc(
        "TRN2",
        target_bir_lowering=False,
        debug=False,
        num_devices=NCORES,
        num_swdge_queues=4,
        dynamic_dma_scratch_size=2048 * KCH,
    )
    xT = nc.dram_tensor("xT", [H + 1, shard_pad], F16, kind="ExternalInput").ap()
    idx_d = nc.dram_tensor("idx", [128, 8 * sumK], I16, kind="ExternalInput").ap()
    w4_d = nc.dram_tensor("w4", [128, 4 * sumK], F16, kind="ExternalInput").ap()
    w1_d = nc.dram_tensor("w1", [H + 1, H], F16, kind="ExternalInput").ap()
    # (Wih @ Wnn_l)^T stacked: [H*L, 3H]; whh per layer with a combined bias
    # row (b_hh + Wih@b_nn_l + b_ih) riding xf's ones row: [(H+1)*L, 3H]
    wihp_d = nc.dram_tensor("wihp", [LAYERS * H, 3 * H], F16, kind="ExternalInput").ap()
    whh_d = nc.dram_tensor("whh", [LAYERS * (H + 1), 3 * H], F16, kind="ExternalInput").ap()
    bias_d = nc.dram_tensor("bias", [H, LAYERS], F32, kind="ExternalInput").ap()
    wd_d = nc.dram_tensor("wd", [H, 1], F16, kind="ExternalInput").ap()
    db_d = nc.dram_tensor("db", [128, 2], F32, kind="ExternalInput").ap()
    out_d = nc.dram_tensor("out", [128, NCLS * panels], F32, kind="ExternalOutput").ap()

    shard_buf = nc.dram_tensor("shard_buf", [shard_pad, H], F16).ap()
    tables = [
        nc.dram_tensor(f"table{i}", [tabrows, 128], F16, addr_space="Shared").ap()
        for i in range(LAYERS)
    ]

    # feature-major x_first per layer lives in DRAM (fp16); gather indices /
    # weight planes stay resident in SBUF
    xf = [nc.dram_tensor(f"xf{i}", [H + 1, shard_pad], F16).ap() for i in range(2)]
    idx_sb = nc.alloc_sbuf_tensor("idx_sb", [128, 8 * sumK], I16).ap()
    w4_sb = nc.alloc_sbuf_tensor("w4_sb", [128, 4 * sumK], F16).ap()

    cc_sem_cm = nc.semaphore("cc_sem")
    cc_sem = cc_sem_cm.__enter__()

    col0i = np.zeros(panels + 1, dtype=np.int64)
    col0i[1:] = np.cumsum(8 * K_uni)
    col0w = np.zeros(panels + 1, dtype=np.int64)
    col0w[1:] = np.cumsum(4 * K_uni)

    # ---------------- TC1: h1 = relu(x @ W1 + b1) for own shard ----------
    with tile.TileContext(nc) as tc:
        with (
            tc.tile_pool(name="cp", bufs=2) as cp,
            tc.tile_pool(name="sp1", bufs=2) as sp1,
            tc.tile_pool(name="const1", bufs=1) as cst,
            tc.tile_pool(name="pp1", bufs=2, space="PSUM") as pp1,
        ):
            ident32 = cst.tile([H, H], F16, tag="id32")
            make_identity(nc, ident32[:])
            w1t = cst.tile([H + 1, H], F16, tag="w1t")
            nc.sync.dma_start(out=w1t[:], in_=w1_d[:])
            ones_t = cst.tile([1, shard_pad], F16, tag="ones")
            nc.vector.memset(ones_t[:], 1.0)
            nc.sync.dma_start(out=xf[0][H : H + 1, :], in_=ones_t[:])
            nc.sync.dma_start(out=xf[1][H : H + 1, :], in_=ones_t[:])
            nc.sync.dma_start(out=idx_sb[:], in_=idx_d[:])
            nc.sync.dma_start(out=w4_sb[:], in_=w4_d[:])
            for b in range(batches):
                cols = slice(512 * b, 512 * (b + 1))
                xt = cp.tile([H + 1, 512], F16, tag="xt")
                nc.sync.dma_start(out=xt[:], in_=xT[:, cols])
                ps = pp1.tile([H, 512], F32, tag="ps")
                nc.tensor.matmul(out=ps[:], lhsT=w1t[:], rhs=xt[:], start=True, stop=True)
                h1 = cp.tile([H, 512], F16, tag="h1")
                nc.scalar.activation(h1[:], ps[:], AF.Relu)
                nc.sync.dma_start(out=xf[0][0:H, cols], in_=h1[:])
                tp = pp1.tile([128, 128], F16, tag="tp")
                for j in range(4):
                    nc.tensor.transpose(
                        out=tp[:, 32 * j : 32 * (j + 1)],
                        in_=h1[:, 128 * j : 128 * (j + 1)],
                        identity=ident32[:],
                    )
                hfp = sp1.tile([128, 128], F16, tag="hfp")
                nc.vector.tensor_copy(out=hfp[:], in_=tp[:])
                nc.sync.dma_start(
                    out=shard_buf[cols, :].rearrange("(j q) f -> q j f", q=128),
                    in_=hfp[:],
                )

    rg = [list(range(NCORES))]
    nc.all_engine_barrier()
    nc.gpsimd.collective_compute(
        "AllGather", ALU.bypass, replica_groups=rg,
        ins=[shard_buf.rearrange("a b -> (a b)")],
        outs=[tables[0].rearrange("a b -> (a b)")],
    ).then_inc(cc_sem, 1)
    nc.gpsimd.wait_ge(cc_sem, 1)
    nc.all_engine_barrier()

    # ---------------- layers ----------------
    call_q = [0]

    def build_layer(li):
        last = li == LAYERS - 1
        with tile.TileContext(nc) as tc:
            with (
                tc.tile_pool(name="gp", bufs=7) as gp,
                tc.tile_pool(name="mp", bufs=4) as mp,
                tc.tile_pool(name="ag", bufs=3) as ag,
                tc.tile_pool(name="sp", bufs=3) as sp,
                tc.tile_pool(name="const2", bufs=1) as cst,
                tc.tile_pool(name="pa", bufs=2, space="PSUM") as pa,
                tc.tile_pool(name="pp", bufs=2, space="PSUM") as pp,
                tc.tile_pool(name="pg", bufs=2, space="PSUM") as pg,
            ):
                ident128 = cst.tile([128, 128], F16, tag="id128")
                make_identity(nc, ident128[:])
                ident32 = cst.tile([H, H], F16, tag="id32")
                make_identity(nc, ident32[:])
                wihp_t = cst.tile([H, 3 * H], F16, tag="wihp")
                nc.sync.dma_start(
                    out=wihp_t[:], in_=wihp_d[li * H : (li + 1) * H, :]
                )
                whh_t = cst.tile([H + 1, 3 * H], F16, tag="whh")
                nc.sync.dma_start(
                    out=whh_t[:], in_=whh_d[li * (H + 1) : (li + 1) * (H + 1), :]
                )
                bias_t = cst.tile([H, LAYERS], F32, tag="bias")
                nc.sync.dma_start(out=bias_t[:], in_=bias_d[:])
                if last:
                    wd_t = cst.tile([H, 1], F16, tag="wd")
                    nc.sync.dma_start(out=wd_t[:], in_=wd_d[:])
                    db_t = cst.tile([128, 2], F32, tag="db")
                    nc.sync.dma_start(out=db_t[:], in_=db_d[:])
                    d_sb = cst.tile([128, panels], F32, tag="dsb")

                table = tables[li]
                for b in range(batches):
                    cols = slice(512 * b, 512 * (b + 1))
                    aggT = pa.tile([H, 512], F32, tag="aggT")
                    agg16 = ag.tile([H, 512], F16, tag="agg16")
                    for pj in range(4):
                        p = 4 * b + pj
                        K = int(K_uni[p])
                        pc = slice(128 * pj, 128 * (pj + 1))
                        if K == 0:
                            nc.vector.memset(agg16[:, pc], 0.0)
                            continue
                        gt = gp.tile([128, K, 128], F16, tag="gt")
                        for a in range(-(-K // KCH)):
                            kk = min(KCH, K - KCH * a)
                            ic = int(col0i[p]) + 8 * KCH * a
                            nc.gpsimd.dma_gather(
                                out_ap=gt[:, KCH * a : KCH * a + kk, :],
                                in_ap=table[:],
                                idxs_ap=idx_sb[:, ic : ic + 8 * kk],
                                num_idxs=128 * kk,
                                num_idxs_reg=128 * kk,
                                elem_size=128,
                                queue_num=call_q[0] % 4,
                            )
                            call_q[0] += 1
                        # weight multiply (w broadcast over features; 1x mode)
                        msg = mp.tile([128, K, 128], F16, tag="msg")
                        wc = int(col0w[p])
                        nc.vector.tensor_tensor(
                            out=msg[:].rearrange("p k (j f) -> p (k j) f", j=4),
                            in0=gt[:].rearrange("p k (j f) -> p (k j) f", j=4),
                            in1=w4_sb[:, wc : wc + 4 * K, None].to_broadcast(
                                [128, 4 * K, H]
                            ),
                            op=ALU.mult,
                        )
                        # slot fold: fused add-tree at 4x DVE mode, down to
                        # <=2 slots; the final slot+j fold happens on the
                        # Tensor engine as PSUM-accumulating identity matmuls
                        # (transpose + j-sum in one accumulation group)
                        kcur = K
                        while kcur > 2:
                            ha = kcur // 2
                            nc.vector.scalar_tensor_tensor(
                                out=msg[:, 0:ha, :],
                                in0=msg[:, 0:ha, :],
                                scalar=1.0,
                                in1=msg[:, ha : 2 * ha, :],
                                op0=ALU.mult,
                                op1=ALU.add,
                            )
                            if kcur % 2 == 1:
                                nc.vector.scalar_tensor_tensor(
                                    out=msg[:, 0:1, :],
                                    in0=msg[:, 0:1, :],
                                    scalar=1.0,
                                    in1=msg[:, 2 * ha : 2 * ha + 1, :],
                                    op0=ALU.mult,
                                    op1=ALU.add,
                                )
                            kcur = ha
                        nmm = 4 * kcur
                        i = 0
                        for k2 in range(kcur):
                            for j in range(4):
                                nc.tensor.matmul(
                                    out=aggT[:, pc],
                                    lhsT=msg[:, k2, 32 * j : 32 * (j + 1)],
                                    rhs=ident128[:],
                                    start=(i == 0),
                                    stop=(i == nmm - 1),
                                )
                                i += 1
                        nc.scalar.activation(agg16[:, pc], aggT[:, pc], AF.Copy)

                    # ---- node phase: per-gate matmuls, all at partitions 0:H;
                    # PSUM accumulation sums gi+gh, bias rows ride the ones rows
                    xfb = sp.tile([H + 1, 512], F16, tag="xfb")
                    nc.sync.dma_start(out=xfb[:], in_=xf[li][:, cols])
                    s_r = pp.tile([H, 512], F32, tag="s_r", bufs=1)
                    nc.tensor.matmul(out=s_r[:], lhsT=wihp_t[:, 0:H], rhs=agg16[:], start=True, stop=False)
                    nc.tensor.matmul(out=s_r[:], lhsT=whh_t[:, 0:H], rhs=xfb[:], start=False, stop=True)
                    s_z = pp.tile([H, 512], F32, tag="s_z", bufs=1)
                    nc.tensor.matmul(out=s_z[:], lhsT=wihp_t[:, H : 2 * H], rhs=agg16[:], start=True, stop=False)
                    nc.tensor.matmul(out=s_z[:], lhsT=whh_t[:, H : 2 * H], rhs=xfb[:], start=False, stop=True)
                    g_in = pp.tile([H, 512], F32, tag="g_in", bufs=1)
                    nc.tensor.matmul(out=g_in[:], lhsT=wihp_t[:, 2 * H : 3 * H], rhs=agg16[:], start=True, stop=True)
                    g_hn = pp.tile([H, 512], F32, tag="g_hn", bufs=1)
                    nc.tensor.matmul(out=g_hn[:], lhsT=whh_t[:, 2 * H : 3 * H], rhs=xfb[:], start=True, stop=True)
                    r16 = sp.tile([H, 512], F16, tag="r16")
                    nc.scalar.activation(r16[:], s_r[:], AF.Sigmoid)
                    z16 = sp.tile([H, 512], F16, tag="z16")
                    nc.scalar.activation(z16[:], s_z[:], AF.Sigmoid)
                    # t1 = g_hn * r ; t1 += g_in ; n = tanh(t1)
                    t1 = sp.tile([H, 512], F16, tag="t1")
                    nc.vector.tensor_tensor(
                        out=t1[:], in0=g_hn[:], in1=r16[:], op=ALU.mult
                    )
                    nc.vector.scalar_tensor_tensor(
                        out=t1[:], in0=g_in[:], scalar=bias_t[:, li : li + 1],
                        in1=t1[:], op0=ALU.add, op1=ALU.add,
                    )
                    n_t = sp.tile([H, 512], F16, tag="nt")
                    nc.scalar.activation(n_t[:], t1[:], AF.Tanh)
                    # t2 = xf - n ; t2 *= z ; h = n + t2 ; ho = fuse*xf + h
                    t2 = sp.tile([H, 512], F16, tag="t2")
                    nc.vector.scalar_tensor_tensor(
                        out=t2[:], in0=n_t[:], scalar=-1.0, in1=xfb[0:H, :],
                        op0=ALU.mult, op1=ALU.add,
                    )
                    nc.vector.scalar_tensor_tensor(
                        out=t2[:], in0=t2[:], scalar=1.0, in1=z16[:],
                        op0=ALU.mult, op1=ALU.mult,
                    )
                    nc.vector.scalar_tensor_tensor(
                        out=t2[:], in0=t2[:], scalar=1.0, in1=n_t[:],
                        op0=ALU.mult, op1=ALU.add,
                    )
                    ho = sp.tile([H, 512], F16, tag="ho")
                    nc.vector.scalar_tensor_tensor(
                        out=ho[:], in0=xfb[0:H, :], scalar=float(fuse[li]), in1=t2[:],
                        op0=ALU.mult, op1=ALU.add,
                    )

                    if not last:
                        nc.sync.dma_start(out=xf[li + 1][0:H, cols], in_=ho[:])
                        tp = pg.tile([128, 128], F16, tag="tp")
                        for j in range(4):
                            nc.tensor.transpose(
                                out=tp[:, 32 * j : 32 * (j + 1)],
                                in_=ho[:, 128 * j : 128 * (j + 1)],
                                identity=ident32[:],
                            )
                        hfp = sp.tile([128, 128], F16, tag="hfp")
                        nc.vector.tensor_copy(out=hfp[:], in_=tp[:])
                        nc.sync.dma_start(
                            out=shard_buf[cols, :].rearrange("(j q) f -> q j f", q=128),
                            in_=hfp[:],
                        )
                    else:
                        lps = pg.tile([128, 4], F32, tag="lps")
                        for j in range(4):
                            nc.tensor.matmul(
                                out=lps[:, j : j + 1],
                                lhsT=ho[:, 128 * j : 128 * (j + 1)],
                                rhs=wd_t[:], start=True, stop=True,
                            )
                        nc.vector.tensor_copy(out=d_sb[:, 4 * b : 4 * b + 4], in_=lps[:])

                if last:
                    # log_softmax, 2 classes: dd = d + db;
                    # out0 = -softplus(dd) = -(relu(dd) + ln(1+exp(-|dd|)))
                    # out1 = -softplus(-dd) = -(relu(-dd) + ln(1+exp(-|dd|)))
                    dd = sp.tile([128, panels], F32, tag="dd")
                    nc.vector.tensor_scalar(
                        out=dd[:], in0=d_sb[:], scalar1=db_t[:, 0:1],
                        scalar2=None, op0=ALU.add,
                    )
                    rp = sp.tile([128, panels], F32, tag="rp")
                    nc.scalar.activation(rp[:], dd[:], AF.Relu)
                    rn = sp.tile([128, panels], F32, tag="rn")
                    nc.scalar.activation(rn[:], dd[:], AF.Relu, scale=-1.0)
                    sab = sp.tile([128, panels], F32, tag="sab")
                    nc.vector.tensor_tensor(out=sab[:], in0=rp[:], in1=rn[:], op=ALU.add)
                    ee = sp.tile([128, panels], F32, tag="ee")
                    nc.scalar.activation(ee[:], sab[:], AF.Exp, scale=-1.0)
                    nc.vector.tensor_scalar(
                        out=ee[:], in0=ee[:], scalar1=1.0, scalar2=None, op0=ALU.add
                    )
                    l1p = sp.tile([128, panels], F32, tag="l1p")
                    nc.scalar.activation(l1p[:], ee[:], AF.Ln)
                    ou = sp.tile([128, panels, NCLS], F32, tag="ou")
                    nc.vector.tensor_tensor(out=rp[:], in0=rp[:], in1=l1p[:], op=ALU.add)
                    nc.vector.tensor_tensor(out=rn[:], in0=rn[:], in1=l1p[:], op=ALU.add)
                    nc.vector.tensor_scalar(
                        out=ou[:, :, 0:1], in0=rp[:, :, None], scalar1=-1.0,
                        scalar2=None, op0=ALU.mult,
                    )
                    nc.vector.tensor_scalar(
                        out=ou[:, :, 1:2], in0=rn[:, :, None], scalar1=-1.0,
                        scalar2=None, op0=ALU.mult,
                    )
                    nc.sync.dma_start(
                        out=out_d[:], in_=ou[:].rearrange("p n c -> p (n c)")
                    )

    build_layer(0)
    nc.all_engine_barrier()
    nc.gpsimd.collective_compute(
        "AllGather", ALU.bypass, replica_groups=rg,
        ins=[shard_buf.rearrange("a b -> (a b)")],
        outs=[tables[1].rearrange("a b -> (a b)")],
    ).then_inc(cc_sem, 1)
    nc.gpsimd.wait_ge(cc_sem, 2)
    nc.all_engine_barrier()
    build_layer(1)

    nc.compile()
    _split_multiwaits(nc)
    cc_sem_cm.__exit__(None, None, None)
    _BUILD_CACHE[key] = nc
    return nc


def _prepare(x, edge_index, edge_weight, W_first, b_first, W_nn, b_nn,
             W_ih, b_ih, W_hh, b_hh, fuse_weight, W_out, b_out):
    shard, shard_pad, panels, tabrows = _sizes(N)
    pre = _preprocess(edge_index, edge_weight)
    order = pre["order"]
    fuse = np.asarray(fuse_weight, np.float32)

    nc = _build(pre["K_uni"], fuse)

    x = np.asarray(x, np.float32)
    W_first = np.asarray(W_first, np.float32)
    b_first = np.asarray(b_first, np.float32)
    W_nn = np.asarray(W_nn, np.float32)
    b_nn = np.asarray(b_nn, np.float32)
    W_ih = np.asarray(W_ih, np.float32)
    b_ih = np.asarray(b_ih, np.float32)
    W_hh = np.asarray(W_hh, np.float32)
    b_hh = np.asarray(b_hh, np.float32)
    W_out = np.asarray(W_out, np.float32)
    b_out = np.asarray(b_out, np.float32)

    w1 = np.concatenate([W_first.T, b_first[None, :]], 0).astype(np.float16)
    # fold W_nn into W_ih: gi = (Wih@Wnn) agg + (Wih@b_nn + b_ih);
    # all biases ride xf's ones row via whh's per-layer bias row
    wihp = np.concatenate(
        [(W_ih @ W_nn[l]).T for l in range(LAYERS)], 0
    ).astype(np.float16)                      # [H*L, 3H]
    whh_rows = []
    bias = np.zeros((H, LAYERS), np.float32)
    for l in range(LAYERS):
        b_ihp = W_ih @ b_nn[l] + b_ih
        brow = b_hh.copy()
        brow[0 : 2 * H] += b_ihp[0 : 2 * H]   # r/z biases additive pre-sigmoid
        bias[:, l] = b_ihp[2 * H : 3 * H]     # n-gate i-side bias: outside r*
        whh_rows.append(np.concatenate([W_hh.T, brow[None, :]], 0))
    whh = np.concatenate(whh_rows, 0).astype(np.float16)  # [(H+1)*L, 3H]
    wd = (W_out[1] - W_out[0]).astype(np.float16)[:, None]  # [H, 1]
    dbv = float(b_out[1] - b_out[0])
    db = np.zeros((128, 2), np.float32)
    db[:, 0] = dbv
    db[:, 1] = -dbv

    in_maps = []
    for c in range(NCORES):
        ids = order[c * shard : (c + 1) * shard]
        xs = np.zeros((H + 1, shard_pad), np.float16)
        xs[0:H, 0:shard] = x[ids].T.astype(np.float16)
        xs[H, :] = 1.0
        in_maps.append(
            {
                "xT": xs,
                "idx": pre["idx_imgs"][c],
                "w4": pre["w4_imgs"][c],
                "w1": w1,
                "wihp": wihp,
                "whh": whh,
                "bias": bias,
                "wd": wd,
                "db": db,
            }
        )

    return nc, in_maps, order


def _assemble(order, results):
    shard, shard_pad, panels, tabrows = _sizes(N)
    out = np.zeros((N, NCLS), np.float32)
    for c in range(NCORES):
        R = np.asarray(results[c]["out"])  # [128, 2*panels]
        R = R.reshape(128, panels, NCLS).transpose(1, 0, 2).reshape(-1, NCLS)
        ids = order[c * shard : (c + 1) * shard]
        out[ids] = R[0:shard]
    return out


def kernel(**inputs):
    nc, in_maps, order = _prepare(**inputs)
    res = run_bass_kernel_spmd(nc, in_maps, core_ids=list(range(NCORES)))
    return _assemble(order, res.results)
